# revision 3
# baseline (speedup 1.0000x reference)
"""Trainium2 Bass kernel for nn_CommAgent (GRU + neighbor-diffusion comm net).

Strategy: data-parallel over 8 NeuronCores (8192 rows = 256 agent-groups per
core), weights replicated. Feature-major activation layout so every GEMM keeps
a [128,128] weight tile stationary and streams 512 rows as the moving operand.
The per-batch 32x32 diffusion runs as: PE-transpose h to row-major, then
matmul(lhsT=h_rowmajor, rhs=blockdiag(nb^T)) which yields c in feature-major
form directly. Neighbor matrices are row-normalized on the host (folds the
/n_neighbor divide) and packed block-diagonally, 4 batches per 128x128 tile.
"""

import numpy as np

try:
    import concourse.bass as bass  # noqa: F401
except ImportError:  # pragma: no cover
    import sys

    sys.path.insert(0, "/opt/trn_rl_repo")

import concourse.bass as bass
import concourse.tile as tile
from concourse import bacc, mybir
from concourse.bass_utils import run_bass_kernel_spmd
from concourse.masks import make_identity

# Problem constants (hardcoded per harness contract)
A = 32          # agents per group
H = 256         # hidden dim
IN = 512        # input feature dim
NA = 16         # n_actions
ROWS = 65536    # total rows = 2048 batches * 32 agents
NCORES = 8
RPC = ROWS // NCORES         # rows per core = 8192
CHUNK = 512                  # rows per fused chunk (16 batches)
NCHUNK = RPC // CHUNK        # 16
NGRP = CHUNK // 128          # row-groups of 128 (4 batches) per chunk
COMM_STEPS = 4
NB_START = 260               # MOVE_FEATS + ENEMY_FEATS
NB_STRIDE = 8                # ALLY_FEATS // (A-1)

F32 = mybir.dt.float32
AluOp = mybir.AluOpType
ActFn = mybir.ActivationFunctionType

_CACHE = {}


def _gru(nc, pg, tmp, act, wih, whh, bias, x_src, h_src, h_out, mm_dt):
    """Emit one GRU cell in feature-major layout.

    x_src, h_src: SBUF [128, 2, CHUNK] (mm dtype). h_out: SBUF [128, 2, CHUNK].
    wih/whh: SBUF [128, 2, 768]. bias: SBUF f32 [128, 8] with columns
    0-3 = (bih+bhh)[r,z], 4-5 = bih[n], 6-7 = bhh[n].
    Computes h_out = (1-z)*n + z*h_src with gates per torch GRUCell.
    """
    r_sb = act.tile([128, 2, CHUNK], mm_dt, tag="r")
    z_sb = act.tile([128, 2, CHUNK], mm_dt, tag="z")
    # r and z gates: accumulate gi+gh in one PSUM bank, sigmoid with summed bias
    for m in range(4):  # gate-feature tiles: r0 r1 z0 z1
        g_ps = pg.tile([128, CHUNK], F32, tag="g")
        for kk in range(2):
            nc.tensor.matmul(
                g_ps[:], wih[:, kk, m * 128:(m + 1) * 128], x_src[:, kk, :],
                start=(kk == 0), stop=False)
        for kk in range(2):
            nc.tensor.matmul(
                g_ps[:], whh[:, kk, m * 128:(m + 1) * 128], h_src[:, kk, :],
                start=False, stop=(kk == 1))
        dst = (r_sb if m < 2 else z_sb)[:, m % 2, :]
        nc.scalar.activation(dst, g_ps[:], ActFn.Sigmoid, bias=bias[:, m:m + 1])
    # n gate + blend, per feature tile
    for t in range(2):
        ghn_ps = pg.tile([128, CHUNK], F32, tag="g")
        for kk in range(2):
            nc.tensor.matmul(
                ghn_ps[:], whh[:, kk, (4 + t) * 128:(5 + t) * 128], h_src[:, kk, :],
                start=(kk == 0), stop=(kk == 1))
        gin_ps = pg.tile([128, CHUNK], F32, tag="g")
        for kk in range(2):
            nc.tensor.matmul(
                gin_ps[:], wih[:, kk, (4 + t) * 128:(5 + t) * 128], x_src[:, kk, :],
                start=(kk == 0), stop=(kk == 1))
        t_sb = tmp.tile([128, CHUNK], mm_dt, tag="t")
        # t = (gh_n + bhh_n) * r
        nc.vector.scalar_tensor_tensor(
            out=t_sb[:], in0=ghn_ps[:], scalar=bias[:, 6 + t:7 + t],
            in1=r_sb[:, t, :], op0=AluOp.add, op1=AluOp.mult)
        u_sb = tmp.tile([128, CHUNK], F32, tag="u")
        # u = (gi_n + bih_n) + t
        nc.vector.scalar_tensor_tensor(
            out=u_sb[:], in0=gin_ps[:], scalar=bias[:, 4 + t:5 + t],
            in1=t_sb[:], op0=AluOp.add, op1=AluOp.add)
        n_sb = tmp.tile([128, CHUNK], mm_dt, tag="n")
        nc.scalar.activation(n_sb[:], u_sb[:], ActFn.Tanh)
        # h' = n + z*(h_src - n)
        v_sb = tmp.tile([128, CHUNK], mm_dt, tag="v")
        nc.gpsimd.tensor_sub(v_sb[:], h_src[:, t, :], n_sb[:])
        w_sb = tmp.tile([128, CHUNK], mm_dt, tag="w")
        nc.vector.tensor_mul(w_sb[:], v_sb[:], z_sb[:, t, :])
        nc.gpsimd.tensor_add(h_out[:, t, :], n_sb[:], w_sb[:])


def build_bass(mm_dt=mybir.dt.float16, loop_n=1):
    """Build the per-core Bass program. loop_n>1 wraps the whole chunk loop in
    a hardware For_i that re-runs the identical computation (for timing)."""
    nc = bacc.Bacc("TRN2", target_bir_lowering=False, debug=False)
    np_mm = mybir.dt.np(mm_dt)

    in_t = nc.dram_tensor("in_t", [IN, RPC], mm_dt, kind="ExternalInput")
    h0_t = nc.dram_tensor("h0_t", [H, RPC], mm_dt, kind="ExternalInput")
    nbt = nc.dram_tensor("nbt", [NCHUNK * NGRP, 128, 128], mm_dt, kind="ExternalInput")
    w1t = nc.dram_tensor("w1t", [IN, H], mm_dt, kind="ExternalInput")
    wih_r = nc.dram_tensor("wih_r", [H, 3 * H], mm_dt, kind="ExternalInput")
    whh_r = nc.dram_tensor("whh_r", [H, 3 * H], mm_dt, kind="ExternalInput")
    wih_c = nc.dram_tensor("wih_c", [H, 3 * H], mm_dt, kind="ExternalInput")
    whh_c = nc.dram_tensor("whh_c", [H, 3 * H], mm_dt, kind="ExternalInput")
    w2t = nc.dram_tensor("w2t", [H, NA], mm_dt, kind="ExternalInput")
    b1p = nc.dram_tensor("b1p", [128, 2], F32, kind="ExternalInput")
    brnn = nc.dram_tensor("brnn", [128, 8], F32, kind="ExternalInput")
    bcom = nc.dram_tensor("bcom", [128, 8], F32, kind="ExternalInput")
    b2p = nc.dram_tensor("b2p", [NA, 1], F32, kind="ExternalInput")
    hout = nc.dram_tensor("hout", [H, RPC], mm_dt, kind="ExternalOutput")
    qout = nc.dram_tensor("qout", [NA, RPC], F32, kind="ExternalOutput")

    with tile.TileContext(nc) as tc:
        import contextlib
        with contextlib.ExitStack() as ctx:
            wp = ctx.enter_context(tc.tile_pool(name="w", bufs=1))
            io = ctx.enter_context(tc.tile_pool(name="io", bufs=2))
            act = ctx.enter_context(tc.tile_pool(name="act", bufs=2))
            tmp = ctx.enter_context(tc.tile_pool(name="tmp", bufs=3))
            hp = ctx.enter_context(tc.tile_pool(name="hp", bufs=2))
            trp = ctx.enter_context(tc.tile_pool(name="trp", bufs=2))
            qp = ctx.enter_context(tc.tile_pool(name="qp", bufs=2))
            pg = ctx.enter_context(
                tc.tile_pool(name="pg", bufs=4, space=bass.MemorySpace.PSUM))
            ptr = ctx.enter_context(
                tc.tile_pool(name="ptr", bufs=3, space=bass.MemorySpace.PSUM))
            pq = ctx.enter_context(
                tc.tile_pool(name="pq", bufs=1, space=bass.MemorySpace.PSUM))

            # Resident weights/biases
            w1_sb = wp.tile([128, 4, H], mm_dt)
            for kk in range(4):
                nc.sync.dma_start(w1_sb[:, kk, :], w1t[kk * 128:(kk + 1) * 128, :])
            gru_w = {}
            for name, dram in (("ihr", wih_r), ("hhr", whh_r),
                               ("ihc", wih_c), ("hhc", whh_c)):
                w_sb = wp.tile([128, 2, 3 * H], mm_dt, tag=name)
                for kk in range(2):
                    nc.sync.dma_start(w_sb[:, kk, :], dram[kk * 128:(kk + 1) * 128, :])
                gru_w[name] = w_sb
            w2_sb = wp.tile([128, 2, NA], mm_dt)
            for kk in range(2):
                nc.sync.dma_start(w2_sb[:, kk, :], w2t[kk * 128:(kk + 1) * 128, :])
            b1_sb = wp.tile([128, 2], F32)
            nc.sync.dma_start(b1_sb[:], b1p[:])
            brnn_sb = wp.tile([128, 8], F32)
            nc.sync.dma_start(brnn_sb[:], brnn[:])
            bcom_sb = wp.tile([128, 8], F32)
            nc.sync.dma_start(bcom_sb[:], bcom[:])
            b2_sb = wp.tile([NA, 1], F32)
            nc.sync.dma_start(b2_sb[:], b2p[:])
            ident = wp.tile([128, 128], mm_dt)
            make_identity(nc, ident[:])

            def chunk_body(c):
                # Loads
                in_sb = io.tile([128, 4, CHUNK], mm_dt, tag="in")
                for kk in range(4):
                    nc.sync.dma_start(
                        in_sb[:, kk, :],
                        in_t[kk * 128:(kk + 1) * 128, c * CHUNK:(c + 1) * CHUNK])
                h0_sb = io.tile([128, 2, CHUNK], mm_dt, tag="h0")
                for kk in range(2):
                    nc.sync.dma_start(
                        h0_sb[:, kk, :],
                        h0_t[kk * 128:(kk + 1) * 128, c * CHUNK:(c + 1) * CHUNK])
                nb_sb = io.tile([128, NGRP, 128], mm_dt, tag="nb")
                nc.sync.dma_start(
                    nb_sb[:], nbt[c * NGRP:(c + 1) * NGRP].transpose([1, 0, 2]))

                # x = relu(W1 @ in + b1)  (fused bias+relu on DVE)
                x_sb = act.tile([128, 2, CHUNK], mm_dt, tag="x")
                for t in range(2):
                    x_ps = pg.tile([128, CHUNK], F32, tag="g")
                    for kk in range(4):
                        nc.tensor.matmul(
                            x_ps[:], w1_sb[:, kk, t * 128:(t + 1) * 128],
                            in_sb[:, kk, :], start=(kk == 0), stop=(kk == 3))
                    nc.vector.tensor_scalar(
                        out=x_sb[:, t, :], in0=x_ps[:],
                        scalar1=b1_sb[:, t:t + 1], scalar2=0.0,
                        op0=AluOp.add, op1=AluOp.max)

                # RNN GRU -> h (also the h_rnn output)
                h_sb = hp.tile([128, 2, CHUNK], mm_dt, tag="h")
                _gru(nc, pg, tmp, act, gru_w["ihr"], gru_w["hhr"], brnn_sb,
                     x_sb, h0_sb, h_sb, mm_dt)
                for t in range(2):
                    nc.sync.dma_start(
                        hout[t * 128:(t + 1) * 128, c * CHUNK:(c + 1) * CHUNK],
                        h_sb[:, t, :])

                # Comm steps
                for _s in range(COMM_STEPS):
                    h_r = trp.tile([128, NGRP, H], mm_dt, tag="hr")
                    for g in range(NGRP):
                        for t in range(2):
                            tp = ptr.tile([128, 128], mm_dt, tag="tr")
                            nc.tensor.transpose(
                                tp[:], h_sb[:, t, g * 128:(g + 1) * 128], ident[:])
                            nc.vector.tensor_copy(
                                h_r[:, g, t * 128:(t + 1) * 128], tp[:])
                    c_f = trp.tile([128, 2, CHUNK], mm_dt, tag="cf")
                    for g in range(NGRP):
                        for t in range(2):
                            cp = ptr.tile([128, 128], F32, tag="tr")
                            nc.tensor.matmul(
                                cp[:], h_r[:, g, t * 128:(t + 1) * 128],
                                nb_sb[:, g, :], start=True, stop=True)
                            nc.vector.tensor_copy(
                                c_f[:, t, g * 128:(g + 1) * 128], cp[:])
                    h_new = hp.tile([128, 2, CHUNK], mm_dt, tag="h")
                    _gru(nc, pg, tmp, act, gru_w["ihc"], gru_w["hhc"], bcom_sb,
                         h_sb, c_f, h_new, mm_dt)
                    h_sb = h_new

                # q = W2 @ h + b2
                q_ps = pq.tile([NA, CHUNK], F32, tag="q")
                for kk in range(2):
                    nc.tensor.matmul(q_ps[:], w2_sb[:, kk, :], h_sb[:, kk, :],
                                     start=(kk == 0), stop=(kk == 1))
                q_sb = qp.tile([NA, CHUNK], F32, tag="qs")
                nc.vector.tensor_scalar_add(q_sb[:], q_ps[:], b2_sb[:, 0:1])
                nc.sync.dma_start(
                    qout[:, c * CHUNK:(c + 1) * CHUNK], q_sb[:])

            if loop_n > 1:
                with tc.For_i(0, loop_n, 1):
                    for c in range(NCHUNK):
                        chunk_body(c)
            else:
                for c in range(NCHUNK):
                    chunk_body(c)

    nc.compile()
    return nc, np_mm


def _host_prep(inputs, hidden_state, W1, b1, rnn_Wih, rnn_Whh, rnn_bih,
               rnn_bhh, comm_Wih, comm_Whh, comm_bih, comm_bhh, W2, b2, np_mm):
    inputs = np.asarray(inputs, dtype=np.float32)
    hidden_state = np.asarray(hidden_state, dtype=np.float32)

    # Neighbor matrices: extract, zero-diag scatter, row-normalize, transpose,
    # pack block-diagonally (4 batches per 128x128 tile).
    B = ROWS // A
    nb31 = inputs[:, NB_START:NB_START + 31 * NB_STRIDE:NB_STRIDE].reshape(B, A, A - 1)
    full = np.zeros((B, A, A), np.float32)
    i_idx = np.arange(A)[:, None]
    k_idx = np.arange(A - 1)[None, :]
    cols = np.where(k_idx >= i_idx, k_idx + 1, k_idx)
    full[:, i_idx, cols] = nb31
    nsum = full.sum(-1, keepdims=True)
    nb_norm = (full / nsum).astype(np.float32)          # [B, A, A]
    ngr = B // 4
    bd = np.zeros((ngr, 4, A, 4, A), np.float32)
    nbt_g = nb_norm.transpose(0, 2, 1).reshape(ngr, 4, A, A)  # [g, b, j, i]
    for b in range(4):
        bd[:, b, :, b, :] = nbt_g[:, b]
    bd = bd.reshape(ngr, 128, 128).astype(np_mm)

    in_t = np.ascontiguousarray(inputs.T.astype(np_mm))       # [512, 65536]
    h0_t = np.ascontiguousarray(hidden_state.T.astype(np_mm))  # [256, 65536]

    def packb(v, n):
        return np.ascontiguousarray(v.reshape(n, 128).T.astype(np.float32))

    def gru_bias(bih, bhh):
        out = np.zeros((128, 8), np.float32)
        out[:, 0:4] = packb(bih[:512] + bhh[:512], 4)
        out[:, 4:6] = packb(bih[512:], 2)
        out[:, 6:8] = packb(bhh[512:], 2)
        return out

    common = {
        "w1t": np.ascontiguousarray(np.asarray(W1, np.float32).T.astype(np_mm)),
        "wih_r": np.ascontiguousarray(np.asarray(rnn_Wih, np.float32).T.astype(np_mm)),
        "whh_r": np.ascontiguousarray(np.asarray(rnn_Whh, np.float32).T.astype(np_mm)),
        "wih_c": np.ascontiguousarray(np.asarray(comm_Wih, np.float32).T.astype(np_mm)),
        "whh_c": np.ascontiguousarray(np.asarray(comm_Whh, np.float32).T.astype(np_mm)),
        "w2t": np.ascontiguousarray(np.asarray(W2, np.float32).T.astype(np_mm)),
        "b1p": packb(np.asarray(b1, np.float32), 2),
        "brnn": gru_bias(np.asarray(rnn_bih, np.float32), np.asarray(rnn_bhh, np.float32)),
        "bcom": gru_bias(np.asarray(comm_bih, np.float32), np.asarray(comm_bhh, np.float32)),
        "b2p": np.asarray(b2, np.float32).reshape(NA, 1),
    }
    in_maps = []
    gpc = ngr // NCORES  # 128x128 nb tiles per core
    for c in range(NCORES):
        in_maps.append({
            "in_t": np.ascontiguousarray(in_t[:, c * RPC:(c + 1) * RPC]),
            "h0_t": np.ascontiguousarray(h0_t[:, c * RPC:(c + 1) * RPC]),
            "nbt": np.ascontiguousarray(bd[c * gpc:(c + 1) * gpc]),
            **common,
        })
    return in_maps


def kernel(**inputs):
    mm_dt = mybir.dt.float16
    key = ("main", str(mm_dt))
    if key not in _CACHE:
        _CACHE[key] = build_bass(mm_dt=mm_dt)
    nc, np_mm = _CACHE[key]
    in_maps = _host_prep(np_mm=np_mm, **inputs)
    res = run_bass_kernel_spmd(nc, in_maps, core_ids=list(range(NCORES)))
    q = np.empty((ROWS, NA), np.float32)
    h_rnn = np.empty((ROWS, H), np.float32)
    for c in range(NCORES):
        q[c * RPC:(c + 1) * RPC] = res.results[c]["qout"].T
        h_rnn[c * RPC:(c + 1) * RPC] = res.results[c]["hout"].astype(np.float32).T
    return q, h_rnn


# revision 13
# speedup vs baseline: 12.8310x; 12.8310x over previous
"""Trainium2 Bass kernel for nn_CommAgent (GRU + neighbor-diffusion comm net).

Strategy: data-parallel over 8 NeuronCores (8192 rows = 256 agent-groups per
core), weights replicated. Feature-major activation layout so every GEMM keeps
a [128,128] weight tile stationary and streams 512 rows as the moving operand.
The per-batch 32x32 diffusion runs as: PE-transpose h to row-major, then
matmul(lhsT=h_rowmajor, rhs=blockdiag(nb^T)) which yields c in feature-major
form directly. Neighbor matrices are row-normalized on the host (folds the
/n_neighbor divide) and packed block-diagonally, 4 batches per 128x128 tile.
"""

import numpy as np

try:
    import concourse.bass as bass  # noqa: F401
except ImportError:  # pragma: no cover
    import sys

    sys.path.insert(0, "/opt/trn_rl_repo")

import concourse.bass as bass
import concourse.tile as tile
from concourse import bacc, mybir
from concourse.bass_utils import run_bass_kernel_spmd
from concourse.masks import make_identity

# Problem constants (hardcoded per harness contract)
A = 32          # agents per group
H = 256         # hidden dim
IN = 512        # input feature dim
NA = 16         # n_actions
ROWS = 65536    # total rows = 2048 batches * 32 agents
NCORES = 8
RPC = ROWS // NCORES         # rows per core = 8192
CHUNK = 512                  # rows per fused chunk (16 batches)
NCHUNK = RPC // CHUNK        # 16
NGRP = CHUNK // 128          # row-groups of 128 (4 batches) per chunk
COMM_STEPS = 4
NB_START = 260               # MOVE_FEATS + ENEMY_FEATS
NB_STRIDE = 8                # ALLY_FEATS // (A-1)

F32 = mybir.dt.float32
AluOp = mybir.AluOpType
ActFn = mybir.ActivationFunctionType

_CACHE = {}


def _gru(nc, pg, tmp, act, wih, whh, bias, x_src, h_src, h_out, mm_dt):
    """Emit one GRU cell in feature-major layout.

    x_src, h_src: SBUF [128, 2, CHUNK] (mm dtype). h_out: SBUF [128, 2, CHUNK].
    wih/whh: SBUF [128, 2, 768]. bias: SBUF f32 [128, 8] with columns
    0-3 = (bih+bhh)[r,z], 4-5 = bih[n], 6-7 = bhh[n].
    Computes h_out = (1-z)*n + z*h_src with gates per torch GRUCell.
    """
    r_sb = act.tile([128, 2, CHUNK], mm_dt, tag="r")
    z_sb = act.tile([128, 2, CHUNK], mm_dt, tag="z")
    # r and z gates: accumulate gi+gh in one PSUM bank, sigmoid with summed bias
    for m in range(4):  # gate-feature tiles: r0 r1 z0 z1
        g_ps = pg.tile([128, CHUNK], F32, tag="g")
        for kk in range(2):
            nc.tensor.matmul(
                g_ps[:], wih[:, kk, m * 128:(m + 1) * 128], x_src[:, kk, :],
                start=(kk == 0), stop=False)
        for kk in range(2):
            nc.tensor.matmul(
                g_ps[:], whh[:, kk, m * 128:(m + 1) * 128], h_src[:, kk, :],
                start=False, stop=(kk == 1))
        dst = (r_sb if m < 2 else z_sb)[:, m % 2, :]
        nc.scalar.activation(dst, g_ps[:], ActFn.Sigmoid, bias=bias[:, m:m + 1])
    # n gate + blend, per feature tile
    for t in range(2):
        ghn_ps = pg.tile([128, CHUNK], F32, tag="g")
        for kk in range(2):
            nc.tensor.matmul(
                ghn_ps[:], whh[:, kk, (4 + t) * 128:(5 + t) * 128], h_src[:, kk, :],
                start=(kk == 0), stop=(kk == 1))
        gin_ps = pg.tile([128, CHUNK], F32, tag="g")
        for kk in range(2):
            nc.tensor.matmul(
                gin_ps[:], wih[:, kk, (4 + t) * 128:(5 + t) * 128], x_src[:, kk, :],
                start=(kk == 0), stop=(kk == 1))
        t_sb = tmp.tile([128, CHUNK], mm_dt, tag="t")
        # t = (gh_n + bhh_n) * r
        nc.vector.scalar_tensor_tensor(
            out=t_sb[:], in0=ghn_ps[:], scalar=bias[:, 6 + t:7 + t],
            in1=r_sb[:, t, :], op0=AluOp.add, op1=AluOp.mult)
        u_sb = tmp.tile([128, CHUNK], F32, tag="u")
        # u = (gi_n + bih_n) + t
        nc.vector.scalar_tensor_tensor(
            out=u_sb[:], in0=gin_ps[:], scalar=bias[:, 4 + t:5 + t],
            in1=t_sb[:], op0=AluOp.add, op1=AluOp.add)
        n_sb = tmp.tile([128, CHUNK], mm_dt, tag="n")
        nc.scalar.activation(n_sb[:], u_sb[:], ActFn.Tanh)
        # h' = n + z*(h_src - n)
        v_sb = tmp.tile([128, CHUNK], mm_dt, tag="v")
        nc.vector.tensor_sub(v_sb[:], h_src[:, t, :], n_sb[:])
        w_sb = tmp.tile([128, CHUNK], mm_dt, tag="w")
        nc.vector.tensor_mul(w_sb[:], v_sb[:], z_sb[:, t, :])
        nc.vector.tensor_add(h_out[:, t, :], n_sb[:], w_sb[:])


def build_bass(mm_dt=mybir.dt.float16, loop_n=1):
    """Build the per-core Bass program. loop_n>1 wraps the whole chunk loop in
    a hardware For_i that re-runs the identical computation (for timing)."""
    nc = bacc.Bacc("TRN2", target_bir_lowering=False, debug=False)
    np_mm = mybir.dt.np(mm_dt)

    in_t = nc.dram_tensor("in_t", [IN, RPC], mm_dt, kind="ExternalInput")
    h0_t = nc.dram_tensor("h0_t", [H, RPC], mm_dt, kind="ExternalInput")
    nbt = nc.dram_tensor("nbt", [NCHUNK * NGRP, 128, 128], mm_dt, kind="ExternalInput")
    w1t = nc.dram_tensor("w1t", [IN, H], mm_dt, kind="ExternalInput")
    wih_r = nc.dram_tensor("wih_r", [H, 3 * H], mm_dt, kind="ExternalInput")
    whh_r = nc.dram_tensor("whh_r", [H, 3 * H], mm_dt, kind="ExternalInput")
    wih_c = nc.dram_tensor("wih_c", [H, 3 * H], mm_dt, kind="ExternalInput")
    whh_c = nc.dram_tensor("whh_c", [H, 3 * H], mm_dt, kind="ExternalInput")
    w2t = nc.dram_tensor("w2t", [H, NA], mm_dt, kind="ExternalInput")
    b1p = nc.dram_tensor("b1p", [128, 2], F32, kind="ExternalInput")
    brnn = nc.dram_tensor("brnn", [128, 8], F32, kind="ExternalInput")
    bcom = nc.dram_tensor("bcom", [128, 8], F32, kind="ExternalInput")
    b2p = nc.dram_tensor("b2p", [NA, 1], F32, kind="ExternalInput")
    hout = nc.dram_tensor("hout", [H, RPC], mm_dt, kind="ExternalOutput")
    qout = nc.dram_tensor("qout", [NA, RPC], F32, kind="ExternalOutput")

    with tile.TileContext(nc) as tc:
        import contextlib
        with contextlib.ExitStack() as ctx:
            wp = ctx.enter_context(tc.tile_pool(name="w", bufs=1))
            io = ctx.enter_context(tc.tile_pool(name="io", bufs=3))
            act = ctx.enter_context(tc.tile_pool(name="act", bufs=3))
            tmp = ctx.enter_context(tc.tile_pool(name="tmp", bufs=4))
            hp = ctx.enter_context(tc.tile_pool(name="hp", bufs=3))
            trp = ctx.enter_context(tc.tile_pool(name="trp", bufs=3))
            qp = ctx.enter_context(tc.tile_pool(name="qp", bufs=2))
            pg = ctx.enter_context(
                tc.tile_pool(name="pg", bufs=5, space=bass.MemorySpace.PSUM))
            ptr = ctx.enter_context(
                tc.tile_pool(name="ptr", bufs=2, space=bass.MemorySpace.PSUM))
            pq = ctx.enter_context(
                tc.tile_pool(name="pq", bufs=1, space=bass.MemorySpace.PSUM))

            # Resident weights/biases
            w1_sb = wp.tile([128, 4, H], mm_dt)
            for kk in range(4):
                nc.sync.dma_start(w1_sb[:, kk, :], w1t[kk * 128:(kk + 1) * 128, :])
            gru_w = {}
            for name, dram in (("ihr", wih_r), ("hhr", whh_r),
                               ("ihc", wih_c), ("hhc", whh_c)):
                w_sb = wp.tile([128, 2, 3 * H], mm_dt, tag=name)
                for kk in range(2):
                    nc.sync.dma_start(w_sb[:, kk, :], dram[kk * 128:(kk + 1) * 128, :])
                gru_w[name] = w_sb
            w2_sb = wp.tile([128, 2, NA], mm_dt)
            for kk in range(2):
                nc.sync.dma_start(w2_sb[:, kk, :], w2t[kk * 128:(kk + 1) * 128, :])
            b1_sb = wp.tile([128, 2], F32)
            nc.sync.dma_start(b1_sb[:], b1p[:])
            brnn_sb = wp.tile([128, 8], F32)
            nc.sync.dma_start(brnn_sb[:], brnn[:])
            bcom_sb = wp.tile([128, 8], F32)
            nc.sync.dma_start(bcom_sb[:], bcom[:])
            b2_sb = wp.tile([NA, 1], F32)
            nc.sync.dma_start(b2_sb[:], b2p[:])
            ident = wp.tile([128, 128], mm_dt)
            make_identity(nc, ident[:])

            in_rr = in_t[:].rearrange("(kk p) r -> p kk r", p=128)
            h0_rr = h0_t[:].rearrange("(kk p) r -> p kk r", p=128)

            def chunk_body(c):
                # Loads (one DMA each; AP regroups 128-row blocks to partitions)
                in_sb = io.tile([128, 4, CHUNK], mm_dt, tag="in")
                nc.sync.dma_start(
                    in_sb[:], in_rr[:, :, c * CHUNK:(c + 1) * CHUNK])
                h0_sb = io.tile([128, 2, CHUNK], mm_dt, tag="h0")
                nc.sync.dma_start(
                    h0_sb[:], h0_rr[:, :, c * CHUNK:(c + 1) * CHUNK])
                nb_sb = io.tile([128, NGRP, 128], mm_dt, tag="nb")
                nc.sync.dma_start(
                    nb_sb[:], nbt[c * NGRP:(c + 1) * NGRP].transpose([1, 0, 2]))

                # x = relu(W1 @ in + b1)  (fused bias+relu on DVE)
                x_sb = act.tile([128, 2, CHUNK], mm_dt, tag="x")
                for t in range(2):
                    x_ps = pg.tile([128, CHUNK], F32, tag="g")
                    for kk in range(4):
                        nc.tensor.matmul(
                            x_ps[:], w1_sb[:, kk, t * 128:(t + 1) * 128],
                            in_sb[:, kk, :], start=(kk == 0), stop=(kk == 3))
                    nc.vector.tensor_scalar(
                        out=x_sb[:, t, :], in0=x_ps[:],
                        scalar1=b1_sb[:, t:t + 1], scalar2=0.0,
                        op0=AluOp.add, op1=AluOp.max)

                # RNN GRU -> h (also the h_rnn output)
                h_sb = hp.tile([128, 2, CHUNK], mm_dt, tag="h")
                _gru(nc, pg, tmp, act, gru_w["ihr"], gru_w["hhr"], brnn_sb,
                     x_sb, h0_sb, h_sb, mm_dt)
                for t in range(2):
                    nc.sync.dma_start(
                        hout[t * 128:(t + 1) * 128, c * CHUNK:(c + 1) * CHUNK],
                        h_sb[:, t, :])

                # Comm steps
                for _s in range(COMM_STEPS):
                    # Transpose h to row-major on the PE; 4 transposes share
                    # one PSUM tile, drained by a single batched DVE copy.
                    h_r = trp.tile([128, NGRP, H], mm_dt, tag="hr")
                    for gh in range(2):
                        tp = ptr.tile([128, 2, H], mm_dt, tag="tr")
                        for g2 in range(2):
                            g = 2 * gh + g2
                            for t in range(2):
                                nc.tensor.transpose(
                                    tp[:, g2, t * 128:(t + 1) * 128],
                                    h_sb[:, t, g * 128:(g + 1) * 128],
                                    ident[:])
                        nc.vector.tensor_copy(h_r[:, 2 * gh:2 * gh + 2, :], tp[:])
                    # Diffusion: c^T tile = h_rowmajor^T @ blockdiag(nb^T); the
                    # 4 row-groups of one feature tile land in one PSUM bank,
                    # drained by a single copy.
                    c_f = trp.tile([128, 2, CHUNK], mm_dt, tag="cf")
                    for t in range(2):
                        cp = pg.tile([128, CHUNK], F32, tag="g")
                        for g in range(NGRP):
                            nc.tensor.matmul(
                                cp[:, g * 128:(g + 1) * 128],
                                h_r[:, g, t * 128:(t + 1) * 128],
                                nb_sb[:, g, :], start=True, stop=True)
                        nc.vector.tensor_copy(c_f[:, t, :], cp[:])
                    h_new = hp.tile([128, 2, CHUNK], mm_dt, tag="h")
                    _gru(nc, pg, tmp, act, gru_w["ihc"], gru_w["hhc"], bcom_sb,
                         h_sb, c_f, h_new, mm_dt)
                    h_sb = h_new

                # q = W2 @ h + b2
                q_ps = pq.tile([NA, CHUNK], F32, tag="q")
                for kk in range(2):
                    nc.tensor.matmul(q_ps[:], w2_sb[:, kk, :], h_sb[:, kk, :],
                                     start=(kk == 0), stop=(kk == 1))
                q_sb = qp.tile([NA, CHUNK], F32, tag="qs")
                nc.vector.tensor_scalar_add(q_sb[:], q_ps[:], b2_sb[:, 0:1])
                nc.sync.dma_start(
                    qout[:, c * CHUNK:(c + 1) * CHUNK], q_sb[:])

            if loop_n > 1:
                with tc.For_i(0, loop_n, 1):
                    for c in range(NCHUNK):
                        chunk_body(c)
            else:
                for c in range(NCHUNK):
                    chunk_body(c)

    nc.compile()
    return nc, np_mm


def _host_prep(inputs, hidden_state, W1, b1, rnn_Wih, rnn_Whh, rnn_bih,
               rnn_bhh, comm_Wih, comm_Whh, comm_bih, comm_bhh, W2, b2, np_mm):
    inputs = np.asarray(inputs, dtype=np.float32)
    hidden_state = np.asarray(hidden_state, dtype=np.float32)

    # Neighbor matrices: extract, zero-diag scatter, row-normalize, transpose,
    # pack block-diagonally (4 batches per 128x128 tile).
    B = ROWS // A
    nb31 = inputs[:, NB_START:NB_START + 31 * NB_STRIDE:NB_STRIDE].reshape(B, A, A - 1)
    full = np.zeros((B, A, A), np.float32)
    i_idx = np.arange(A)[:, None]
    k_idx = np.arange(A - 1)[None, :]
    cols = np.where(k_idx >= i_idx, k_idx + 1, k_idx)
    full[:, i_idx, cols] = nb31
    nsum = full.sum(-1, keepdims=True)
    nb_norm = (full / nsum).astype(np.float32)          # [B, A, A]
    ngr = B // 4
    bd = np.zeros((ngr, 4, A, 4, A), np.float32)
    nbt_g = nb_norm.transpose(0, 2, 1).reshape(ngr, 4, A, A)  # [g, b, j, i]
    for b in range(4):
        bd[:, b, :, b, :] = nbt_g[:, b]
    bd = bd.reshape(ngr, 128, 128).astype(np_mm)

    in_t = np.ascontiguousarray(inputs.T.astype(np_mm))       # [512, 65536]
    h0_t = np.ascontiguousarray(hidden_state.T.astype(np_mm))  # [256, 65536]

    def packb(v, n):
        return np.ascontiguousarray(v.reshape(n, 128).T.astype(np.float32))

    def gru_bias(bih, bhh):
        out = np.zeros((128, 8), np.float32)
        out[:, 0:4] = packb(bih[:512] + bhh[:512], 4)
        out[:, 4:6] = packb(bih[512:], 2)
        out[:, 6:8] = packb(bhh[512:], 2)
        return out

    common = {
        "w1t": np.ascontiguousarray(np.asarray(W1, np.float32).T.astype(np_mm)),
        "wih_r": np.ascontiguousarray(np.asarray(rnn_Wih, np.float32).T.astype(np_mm)),
        "whh_r": np.ascontiguousarray(np.asarray(rnn_Whh, np.float32).T.astype(np_mm)),
        "wih_c": np.ascontiguousarray(np.asarray(comm_Wih, np.float32).T.astype(np_mm)),
        "whh_c": np.ascontiguousarray(np.asarray(comm_Whh, np.float32).T.astype(np_mm)),
        "w2t": np.ascontiguousarray(np.asarray(W2, np.float32).T.astype(np_mm)),
        "b1p": packb(np.asarray(b1, np.float32), 2),
        "brnn": gru_bias(np.asarray(rnn_bih, np.float32), np.asarray(rnn_bhh, np.float32)),
        "bcom": gru_bias(np.asarray(comm_bih, np.float32), np.asarray(comm_bhh, np.float32)),
        "b2p": np.asarray(b2, np.float32).reshape(NA, 1),
    }
    in_maps = []
    gpc = ngr // NCORES  # 128x128 nb tiles per core
    for c in range(NCORES):
        in_maps.append({
            "in_t": np.ascontiguousarray(in_t[:, c * RPC:(c + 1) * RPC]),
            "h0_t": np.ascontiguousarray(h0_t[:, c * RPC:(c + 1) * RPC]),
            "nbt": np.ascontiguousarray(bd[c * gpc:(c + 1) * gpc]),
            **common,
        })
    return in_maps


def kernel(**inputs):
    mm_dt = mybir.dt.float16
    key = ("main", str(mm_dt))
    if key not in _CACHE:
        _CACHE[key] = build_bass(mm_dt=mm_dt)
    nc, np_mm = _CACHE[key]
    in_maps = _host_prep(np_mm=np_mm, **inputs)
    res = run_bass_kernel_spmd(nc, in_maps, core_ids=list(range(NCORES)))
    q = np.empty((ROWS, NA), np.float32)
    h_rnn = np.empty((ROWS, H), np.float32)
    for c in range(NCORES):
        q[c * RPC:(c + 1) * RPC] = res.results[c]["qout"].T
        h_rnn[c * RPC:(c + 1) * RPC] = res.results[c]["hout"].astype(np.float32).T
    return q, h_rnn
